# revision 1
# baseline (speedup 1.0000x reference)
"""Trainium2 Bass kernel for nn_Compression.

Computes: out = X + GAMMA * (P @ (P.T @ X)),  P = softmax(X @ W.T + b)

Strategy (8 NeuronCores, data-parallel over N):
  - Each core owns NLOC = N/8 = 4096 rows of X (32 tiles of 128 rows).
  - Phase A per row-tile: cast X tile to bf16, PE-transpose it (the
    logits contraction over D needs D on partitions), logits via bf16
    matmuls (+ b added via a K=1 matmul against a ones vector), softmax
    with fused exp+row-sum on ScalarE, then accumulate P.T @ X into 4
    resident PSUM banks.
  - One AllReduce of the [C, D] = 1 MiB f32 partial.
  - Phase B per row-tile: corr = P @ (gamma * PtX) in bf16, residual add
    against the SBUF-resident f32 X, DMA out.

Precision note: the correction term is scaled by GAMMA=1e-4 while the
residual X passes through in exact f32, so bf16 compute of the
correction contributes ~1e-6 relative error to the output.

The host side only reshapes: shards X rows, passes W transposed (pure
relayout, still f32) and b as-is.
"""

import sys

import numpy as np

if "/opt/trn_rl_repo" not in sys.path:
    sys.path.insert(0, "/opt/trn_rl_repo")

N, D, C = 32768, 1024, 256
GAMMA = 1e-4
NCORES = 8
NLOC = N // NCORES  # 4096
P = 128
NT = NLOC // P  # 32
DH = 512

_cache = {}


def _build_nc():
    import concourse.tile as tile
    from concourse import bacc
    import concourse.mybir as mybir
    from concourse.masks import make_identity
    from contextlib import ExitStack

    f32 = mybir.dt.float32
    bf16 = mybir.dt.bfloat16
    AF = mybir.ActivationFunctionType

    nc = bacc.Bacc("TRN2", target_bir_lowering=False, debug=False, num_devices=NCORES)
    X = nc.dram_tensor("X", [NLOC, D], f32, kind="ExternalInput").ap()
    Wt = nc.dram_tensor("Wt", [D, C], f32, kind="ExternalInput").ap()
    bvec = nc.dram_tensor("b", [C], f32, kind="ExternalInput").ap()
    out = nc.dram_tensor("out", [NLOC, D], f32, kind="ExternalOutput").ap()

    with tile.TileContext(nc) as tc, ExitStack() as ctx:
        const = ctx.enter_context(tc.tile_pool(name="const", bufs=1))
        xres = ctx.enter_context(tc.tile_pool(name="xres", bufs=1))
        # xb lives from load(i+2) to ptx(i-2) -> 4 slots; xt only spans
        # transpose(i+1) -> logits(i) -> 2 suffices
        xbp = ctx.enter_context(tc.tile_pool(name="xbp", bufs=4))
        work = ctx.enter_context(tc.tile_pool(name="work", bufs=2))
        ppool = ctx.enter_context(tc.tile_pool(name="ppool", bufs=4))
        spool = ctx.enter_context(tc.tile_pool(name="spool", bufs=4))
        opool = ctx.enter_context(tc.tile_pool(name="opool", bufs=3))
        dram = ctx.enter_context(tc.tile_pool(name="dram", bufs=1, space="DRAM"))

        ident = const.tile([P, P], bf16)
        make_identity(nc, ident)

        # W.T in bf16, [d-within-chunk, k-chunk, c]. Loaded in 4 parallel
        # DMA chunks and cast on ScalarE so the first X-tile cast on DVE
        # isn't stuck behind it (engine queues are FIFO).
        Wt_sb = const.tile([P, 8, C], bf16)
        with tc.tile_pool(name="wtmp", bufs=1) as wtmp:
            wt_f = wtmp.tile([P, 8, C], f32)
            wt_r = Wt.rearrange("(k p) c -> p k c", p=P)
            for q in range(4):
                nc.sync.dma_start(wt_f[:, 2 * q:2 * q + 2, :], wt_r[:, 2 * q:2 * q + 2, :])
                nc.scalar.copy(Wt_sb[:, 2 * q:2 * q + 2, :], wt_f[:, 2 * q:2 * q + 2, :])

        ones1 = const.tile([1, P], bf16)
        nc.vector.memset(ones1[:], 1.0)
        b_sb = const.tile([1, C], bf16)
        with tc.tile_pool(name="btmp", bufs=1) as btmp:
            b_f = btmp.tile([1, C], f32)
            nc.sync.dma_start(b_f[:], bvec.rearrange("(o c) -> o c", o=1))
            nc.vector.tensor_copy(b_sb[:], b_f[:])

        Xall = xres.tile([P, NT, D], f32)
        Pt = const.tile([P, 2, NLOC], bf16)  # P.T resident, bf16

        # AllReduce split into two D-halves so the second half overlaps
        # phase-B compute on the first.
        ar_in = [dram.tile([C, DH], f32, name=f"ar_in{h}") for h in range(2)]
        ar_out = [
            dram.tile([C, DH], f32, addr_space="Shared", name=f"ar_out{h}")
            for h in range(2)
        ]

        # ---- phase A: software-pipelined over row-tiles ----
        # Per step i the PE stream is: logits(i), transposes(i+1),
        # PtX/PT(i-1). The softmax ACT->DVE round-trip for tile i then
        # hides under transposes(i+1) + PtX(i-1), and the transpose-copy
        # (ACT) for i+1 hides under PtX(i-1) + logits(i+1) -- no PE idle,
        # which also keeps the HAM clock-gate at full rate.
        def s_load(i):
            xi = Xall[:, i, :]
            nc.sync.dma_start(xi, X[i * P:(i + 1) * P, :])
            xb = xbp.tile([P, D], bf16, name="xb", tag="xb")
            nc.vector.tensor_copy(xb[:], xi)
            return xb

        def s_transpose(i, xb):
            # 8 PE transposes into one PSUM bank as a single accumulation
            # group (start clears the whole bank once).
            xt = work.tile([P, D], bf16, name="xt", tag="xt")
            trp = psA.tile([P, D], bf16, name="trp", tag="trp")
            for k in range(8):
                nc.tensor.matmul(
                    trp[:, k * P:(k + 1) * P],
                    xb[:, k * P:(k + 1) * P],
                    ident[:],
                    is_transpose=True,
                    start=(k == 0),
                    stop=(k == 7),
                )
            nc.scalar.copy(xt[:], trp[:])
            return xt

        def s_logits(i, xt):
            lg = psL.tile([P, C], f32, name="lg", tag="lg")
            for k in range(8):
                nc.tensor.matmul(
                    lg[:],
                    xt[:, k * P:(k + 1) * P],
                    Wt_sb[:, k, :],
                    start=(k == 0),
                    stop=False,
                )
            nc.tensor.matmul(lg[:], ones1[:], b_sb[:], start=False, stop=True)
            return lg

        def s_softmax(i, lg):
            # |logits| <= ~10 so exp is safe without max-subtraction
            p_sb = ppool.tile([P, C], f32, name="p_sb", tag="p")
            ssum = spool.tile([P, 1], f32, name="ssum", tag="s")
            nc.scalar.activation(p_sb[:], lg[:], AF.Exp, accum_out=ssum[:])
            rinv = spool.tile([P, 1], f32, name="rinv", tag="r")
            nc.vector.reciprocal(rinv[:], ssum[:])
            p_bf = ppool.tile([P, C], bf16, name="p_bf", tag="pb")
            nc.vector.tensor_scalar_mul(p_bf[:], p_sb[:], rinv[:])
            return p_bf

        def s_ptx(i, p_bf, xb):
            for c in range(2):
                for h in range(2):
                    nc.tensor.matmul(
                        ptx_ps[2 * c + h][:],
                        p_bf[:, c * P:(c + 1) * P],
                        xb[:, h * DH:(h + 1) * DH],
                        start=(i == 0),
                        stop=(i == NT - 1),
                    )
            ptp = psA.tile([P, C], bf16, name="ptp", tag="trp")
            for c in range(2):
                nc.tensor.matmul(
                    ptp[:, c * P:(c + 1) * P],
                    p_bf[:, c * P:(c + 1) * P],
                    ident[:],
                    is_transpose=True,
                    start=(c == 0),
                    stop=(c == 1),
                )
            nc.scalar.copy(
                Pt[:, :, i * P:(i + 1) * P],
                ptp[:].rearrange("p (c n) -> p c n", c=2),
            )

        with tc.tile_pool(name="psA", bufs=3, space="PSUM") as psA, \
             tc.tile_pool(name="psL", bufs=1, space="PSUM") as psL, \
             tc.tile_pool(name="psX", bufs=1, space="PSUM") as psX:
            ptx_ps = [
                psX.tile([P, DH], f32, name=f"ptx_{c}_{h}", tag=f"ptx_{c}_{h}")
                for c in range(2)
                for h in range(2)
            ]
            # 2-step skew between softmax(i) and ptx(i): the ~1.1us ScalarE
            # exp latency then hides under transposes + the previous ptx +
            # the next logits block instead of stalling the PE.
            xb0 = s_load(0)
            xt0 = s_transpose(0, xb0)
            xb1 = s_load(1)
            state = {0: (xb0, xt0, None), 1: (xb1, None, None)}
            for i in range(NT):
                xb_i, xt_i, _ = state[i]
                lg = s_logits(i, xt_i)
                p_bf = s_softmax(i, lg)
                state[i] = (xb_i, xt_i, p_bf)
                if i + 1 < NT:
                    xb_n, _, _ = state[i + 1]
                    state[i + 1] = (xb_n, s_transpose(i + 1, xb_n), None)
                if i + 2 < NT:
                    state[i + 2] = (s_load(i + 2), None, None)
                if i >= 2:
                    xb_p, _, p_bf_p = state.pop(i - 2)
                    s_ptx(i - 2, p_bf_p, xb_p)
            for i in (NT - 2, NT - 1):
                xb_l, _, p_bf_l = state.pop(i)
                s_ptx(i, p_bf_l, xb_l)

            # PSUM -> SBUF -> DRAM bounce, one per D-half (copies split
            # across DVE and ACT to shorten the pre-collective tail)
            stg = []
            for h in range(2):
                s = const.tile([P, 2, DH], f32, name=f"stg{h}", tag=f"stg{h}")
                nc.vector.tensor_copy(s[:, 0, :], ptx_ps[h][:])
                nc.scalar.copy(s[:, 1, :], ptx_ps[2 + h][:])
                nc.sync.dma_start(
                    ar_in[h].rearrange("(c p) d -> p c d", p=P), s[:]
                )
                stg.append(s)

        # ---- phase B, interleaved with the collectives: AllReduce h=1 is
        # emitted AFTER phase B h=0 so h=0's consumers only wait on the
        # first collective's completion tick, and the second collective
        # runs concurrently with h=0 compute. gamma folded into PtX so the
        # residual is one add. ScalarE drains PSUM->SBUF so the DVE add
        # runs in SBUF-only 2x mode. ----
        def ar(h):
            nc.gpsimd.collective_compute(
                "AllReduce",
                mybir.AluOpType.add,
                replica_groups=[list(range(NCORES))],
                ins=[ar_in[h][:].opt()],
                outs=[ar_out[h][:].opt()],
            )

        def phase_b(h, psB, cpool):
            pall = const.tile([P, 2, DH], f32, name=f"pall{h}", tag=f"stg{h}")
            nc.sync.dma_start(
                pall[:], ar_out[h].rearrange("(c p) d -> p c d", p=P)
            )
            ptxb = const.tile([P, 2, DH], bf16, name=f"ptxb{h}")
            nc.scalar.mul(ptxb[:], pall[:], GAMMA)
            for i in range(NT):
                cor = psB.tile([P, DH], f32, name="cor", tag="cor")
                for c in range(2):
                    nc.tensor.matmul(
                        cor[:],
                        Pt[:, c, i * P:(i + 1) * P],
                        ptxb[:, c, :],
                        start=(c == 0),
                        stop=(c == 1),
                    )
                o_sb = opool.tile([P, DH], f32, name="o_sb", tag="o")
                nc.vector.tensor_add(o_sb[:], cor[:], Xall[:, i, h * DH:(h + 1) * DH])
                nc.sync.dma_start(out[i * P:(i + 1) * P, h * DH:(h + 1) * DH], o_sb[:])

        with tc.tile_pool(name="psB", bufs=8, space="PSUM") as psB:
            ar(0)
            phase_b(0, psB, None)
            ar(1)
            phase_b(1, psB, None)

    nc.finalize()
    return nc


def _run(inputs, trace=False, **kwargs):
    from concourse import bass_utils

    if "nc" not in _cache:
        _cache["nc"] = _build_nc()
    nc = _cache["nc"]

    X = np.ascontiguousarray(np.asarray(inputs["X"], dtype=np.float32))
    W = np.ascontiguousarray(np.asarray(inputs["W"], dtype=np.float32))
    b = np.ascontiguousarray(np.asarray(inputs["b"], dtype=np.float32))
    Wt = np.ascontiguousarray(W.T)

    in_maps = [
        {"X": X[i * NLOC:(i + 1) * NLOC], "Wt": Wt, "b": b} for i in range(NCORES)
    ]
    res = bass_utils.run_bass_kernel_spmd(
        nc, in_maps, core_ids=list(range(NCORES)), trace=trace, **kwargs
    )
    outp = np.concatenate([res.results[i]["out"] for i in range(NCORES)], axis=0)
    return outp, res


def kernel(**inputs):
    outp, _ = _run(inputs, trace=False)
    return outp



# revision 5
# speedup vs baseline: 1.0894x; 1.0894x over previous
"""Trainium2 Bass kernel for nn_Compression.

Computes: out = X + GAMMA * (P @ (P.T @ X)),  P = softmax(X @ W.T + b)

Strategy (8 NeuronCores, data-parallel over N):
  - Each core owns NLOC = N/8 = 4096 rows of X (32 tiles of 128 rows).
  - Phase A per row-tile: cast X tile to bf16, PE-transpose it (the
    logits contraction over D needs D on partitions), logits via bf16
    matmuls (+ b added via a K=1 matmul against a ones vector), softmax
    with fused exp+row-sum on ScalarE, then accumulate P.T @ X into 4
    resident PSUM banks.
  - One AllReduce of the [C, D] = 1 MiB f32 partial.
  - Phase B per row-tile: corr = P @ (gamma * PtX) in bf16, residual add
    against the SBUF-resident f32 X, DMA out.

Precision note: the correction term is scaled by GAMMA=1e-4 while the
residual X passes through in exact f32, so bf16 compute of the
correction contributes ~1e-6 relative error to the output.

The host side only reshapes: shards X rows, passes W transposed (pure
relayout, still f32) and b as-is.
"""

import sys

import numpy as np

if "/opt/trn_rl_repo" not in sys.path:
    sys.path.insert(0, "/opt/trn_rl_repo")

N, D, C = 32768, 1024, 256
GAMMA = 1e-4
NCORES = 8
NLOC = N // NCORES  # 4096
P = 128
NT = NLOC // P  # 32
DH = 512

_cache = {}


def _build_nc():
    import concourse.tile as tile
    from concourse import bacc
    import concourse.mybir as mybir
    from concourse.masks import make_identity
    from contextlib import ExitStack

    f32 = mybir.dt.float32
    bf16 = mybir.dt.bfloat16
    AF = mybir.ActivationFunctionType

    nc = bacc.Bacc("TRN2", target_bir_lowering=False, debug=False, num_devices=NCORES)
    X = nc.dram_tensor("X", [NLOC, D], f32, kind="ExternalInput").ap()
    Wt = nc.dram_tensor("Wt", [D, C], f32, kind="ExternalInput").ap()
    bvec = nc.dram_tensor("b", [C], f32, kind="ExternalInput").ap()
    out = nc.dram_tensor("out", [NLOC, D], f32, kind="ExternalOutput").ap()

    with tile.TileContext(nc) as tc, ExitStack() as ctx:
        const = ctx.enter_context(tc.tile_pool(name="const", bufs=1))
        xres = ctx.enter_context(tc.tile_pool(name="xres", bufs=1))
        # xb lives from load(i+2) to ptx(i-2) -> 4 slots; xt only spans
        # transpose(i+1) -> logits(i) -> 2 suffices
        xbp = ctx.enter_context(tc.tile_pool(name="xbp", bufs=4))
        work = ctx.enter_context(tc.tile_pool(name="work", bufs=2))
        ppool = ctx.enter_context(tc.tile_pool(name="ppool", bufs=4))
        spool = ctx.enter_context(tc.tile_pool(name="spool", bufs=4))
        opool = ctx.enter_context(tc.tile_pool(name="opool", bufs=6))
        dram = ctx.enter_context(tc.tile_pool(name="dram", bufs=1, space="DRAM"))

        ident = const.tile([P, P], bf16)
        make_identity(nc, ident)

        # W.T in bf16, [d-within-chunk, k-chunk, c]. Loaded in 4 parallel
        # DMA chunks and cast on ScalarE so the first X-tile cast on DVE
        # isn't stuck behind it (engine queues are FIFO).
        Wt_sb = const.tile([P, 8, C], bf16)
        with tc.tile_pool(name="wtmp", bufs=1) as wtmp:
            wt_f = wtmp.tile([P, 8, C], f32)
            wt_r = Wt.rearrange("(k p) c -> p k c", p=P)
            for q in range(4):
                nc.sync.dma_start(wt_f[:, 2 * q:2 * q + 2, :], wt_r[:, 2 * q:2 * q + 2, :])
                nc.scalar.copy(Wt_sb[:, 2 * q:2 * q + 2, :], wt_f[:, 2 * q:2 * q + 2, :])

        ones1 = const.tile([1, P], bf16)
        nc.vector.memset(ones1[:], 1.0)
        b_sb = const.tile([1, C], bf16)
        with tc.tile_pool(name="btmp", bufs=1) as btmp:
            b_f = btmp.tile([1, C], f32)
            nc.sync.dma_start(b_f[:], bvec.rearrange("(o c) -> o c", o=1))
            nc.vector.tensor_copy(b_sb[:], b_f[:])

        Xall = xres.tile([P, NT, D], f32)
        Pt = const.tile([P, 2, NLOC], bf16)  # P.T resident, bf16

        # AllReduce split into two D-halves so the second half overlaps
        # phase-B compute on the first.
        ar_in = [dram.tile([C, DH], bf16, name=f"ar_in{h}") for h in range(2)]
        ar_out = [
            dram.tile([C, DH], bf16, addr_space="Shared", name=f"ar_out{h}")
            for h in range(2)
        ]

        # ---- phase A: software-pipelined over row-tiles ----
        # Per step i the PE stream is: logits(i), transposes(i+1),
        # PtX/PT(i-1). The softmax ACT->DVE round-trip for tile i then
        # hides under transposes(i+1) + PtX(i-1), and the transpose-copy
        # (ACT) for i+1 hides under PtX(i-1) + logits(i+1) -- no PE idle,
        # which also keeps the HAM clock-gate at full rate.
        def s_load(i):
            xi = Xall[:, i, :]
            nc.sync.dma_start(xi, X[i * P:(i + 1) * P, :])
            xb = xbp.tile([P, D], bf16, name="xb", tag="xb")
            nc.vector.tensor_copy(xb[:], xi)
            return xb

        def s_transpose(i, xb):
            # 8 PE transposes into one PSUM bank as a single accumulation
            # group (start clears the whole bank once).
            xt = work.tile([P, D], bf16, name="xt", tag="xt")
            trp = psA.tile([P, D], bf16, name="trp", tag="trp")
            for k in range(8):
                nc.tensor.matmul(
                    trp[:, k * P:(k + 1) * P],
                    xb[:, k * P:(k + 1) * P],
                    ident[:],
                    is_transpose=True,
                    start=(k == 0),
                    stop=(k == 7),
                )
            nc.scalar.copy(xt[:], trp[:])
            return xt

        def s_logits(i, xt):
            lg = psL.tile([P, C], f32, name="lg", tag="lg")
            for k in range(8):
                nc.tensor.matmul(
                    lg[:],
                    xt[:, k * P:(k + 1) * P],
                    Wt_sb[:, k, :],
                    start=(k == 0),
                    stop=False,
                )
            nc.tensor.matmul(lg[:], ones1[:], b_sb[:], start=False, stop=True)
            return lg

        def s_softmax(i, lg):
            # |logits| <= ~10 so exp is safe without max-subtraction
            p_sb = ppool.tile([P, C], f32, name="p_sb", tag="p")
            ssum = spool.tile([P, 1], f32, name="ssum", tag="s")
            nc.scalar.activation(p_sb[:], lg[:], AF.Exp, accum_out=ssum[:])
            rinv = spool.tile([P, 1], f32, name="rinv", tag="r")
            nc.vector.reciprocal(rinv[:], ssum[:])
            p_bf = ppool.tile([P, C], bf16, name="p_bf", tag="pb")
            nc.vector.tensor_scalar_mul(p_bf[:], p_sb[:], rinv[:])
            return p_bf

        def s_ptx(i, p_bf, xb):
            for c in range(2):
                for h in range(2):
                    nc.tensor.matmul(
                        ptx_ps[2 * c + h][:],
                        p_bf[:, c * P:(c + 1) * P],
                        xb[:, h * DH:(h + 1) * DH],
                        start=(i == 0),
                        stop=(i == NT - 1),
                    )
            ptp = psA.tile([P, C], bf16, name="ptp", tag="trp")
            for c in range(2):
                nc.tensor.matmul(
                    ptp[:, c * P:(c + 1) * P],
                    p_bf[:, c * P:(c + 1) * P],
                    ident[:],
                    is_transpose=True,
                    start=(c == 0),
                    stop=(c == 1),
                )
            nc.scalar.copy(
                Pt[:, :, i * P:(i + 1) * P],
                ptp[:].rearrange("p (c n) -> p c n", c=2),
            )

        with tc.tile_pool(name="psA", bufs=3, space="PSUM") as psA, \
             tc.tile_pool(name="psL", bufs=1, space="PSUM") as psL, \
             tc.tile_pool(name="psX", bufs=1, space="PSUM") as psX:
            ptx_ps = [
                psX.tile([P, DH], f32, name=f"ptx_{c}_{h}", tag=f"ptx_{c}_{h}")
                for c in range(2)
                for h in range(2)
            ]
            # 2-step skew between softmax(i) and ptx(i): the ~1.1us ScalarE
            # exp latency then hides under transposes + the previous ptx +
            # the next logits block instead of stalling the PE.
            xb0 = s_load(0)
            xt0 = s_transpose(0, xb0)
            xb1 = s_load(1)
            state = {0: (xb0, xt0, None), 1: (xb1, None, None)}
            for i in range(NT):
                xb_i, xt_i, _ = state[i]
                lg = s_logits(i, xt_i)
                p_bf = s_softmax(i, lg)
                state[i] = (xb_i, xt_i, p_bf)
                if i + 1 < NT:
                    xb_n, _, _ = state[i + 1]
                    state[i + 1] = (xb_n, s_transpose(i + 1, xb_n), None)
                if i + 2 < NT:
                    state[i + 2] = (s_load(i + 2), None, None)
                if i >= 2:
                    xb_p, _, p_bf_p = state.pop(i - 2)
                    s_ptx(i - 2, p_bf_p, xb_p)
            for i in (NT - 2, NT - 1):
                xb_l, _, p_bf_l = state.pop(i)
                s_ptx(i, p_bf_l, xb_l)

            # PSUM -> SBUF -> DRAM bounce, one per D-half (copies split
            # across DVE and ACT to shorten the pre-collective tail).
            # Drains cast to bf16: halves the collective bytes; the PtX
            # rounding (2^-9 rel) is gamma-scaled so it is invisible in
            # the output.
            stg = []
            for h in range(2):
                s = const.tile([P, 2, DH], bf16, name=f"stg{h}", tag=f"stg{h}")
                nc.vector.tensor_copy(s[:, 0, :], ptx_ps[h][:])
                nc.scalar.copy(s[:, 1, :], ptx_ps[2 + h][:])
                nc.sync.dma_start(
                    ar_in[h].rearrange("(c p) d -> p c d", p=P), s[:]
                )
                stg.append(s)

        # ---- phase B, interleaved with the collectives: AllReduce h=1 is
        # emitted AFTER phase B h=0 so h=0's consumers only wait on the
        # first collective's completion tick, and the second collective
        # runs concurrently with h=0 compute. gamma folded into PtX so the
        # residual is one add. ScalarE drains PSUM->SBUF so the DVE add
        # runs in SBUF-only 2x mode. ----
        def ar(h):
            nc.gpsimd.collective_compute(
                "AllReduce",
                mybir.AluOpType.add,
                replica_groups=[list(range(NCORES))],
                ins=[ar_in[h][:].opt()],
                outs=[ar_out[h][:].opt()],
            )

        def phase_b(h, psB, cpool):
            pall = const.tile([P, 2, DH], bf16, name=f"pall{h}", tag=f"stg{h}")
            nc.sync.dma_start(
                pall[:], ar_out[h].rearrange("(c p) d -> p c d", p=P)
            )
            ptxb = const.tile([P, 2, DH], bf16, name=f"ptxb{h}")
            nc.vector.tensor_scalar_mul(ptxb[:], pall[:], GAMMA)
            for i in range(NT):
                cor = psB.tile([P, DH], f32, name="cor", tag="cor")
                for c in range(2):
                    nc.tensor.matmul(
                        cor[:],
                        Pt[:, c, i * P:(i + 1) * P],
                        ptxb[:, c, :],
                        start=(c == 0),
                        stop=(c == 1),
                    )
                # ACT drains PSUM->SBUF; DVE then adds SBUF+SBUF so neither
                # engine exceeds the per-tile store budget.
                cors = cpool.tile([P, DH], f32, name="cors", tag="cs")
                nc.scalar.copy(cors[:], cor[:])
                o_sb = opool.tile([P, DH], f32, name="o_sb", tag="o")
                nc.vector.tensor_add(o_sb[:], cors[:], Xall[:, i, h * DH:(h + 1) * DH])
                nc.sync.dma_start(out[i * P:(i + 1) * P, h * DH:(h + 1) * DH], o_sb[:])

        with tc.tile_pool(name="psB", bufs=6, space="PSUM") as psB, \
             tc.tile_pool(name="cpool", bufs=4) as cpool:
            ar(0)
            phase_b(0, psB, cpool)
            ar(1)
            phase_b(1, psB, cpool)

    nc.finalize()
    return nc


def _run(inputs, trace=False, **kwargs):
    from concourse import bass_utils

    if "nc" not in _cache:
        _cache["nc"] = _build_nc()
    nc = _cache["nc"]

    X = np.ascontiguousarray(np.asarray(inputs["X"], dtype=np.float32))
    W = np.ascontiguousarray(np.asarray(inputs["W"], dtype=np.float32))
    b = np.ascontiguousarray(np.asarray(inputs["b"], dtype=np.float32))
    Wt = np.ascontiguousarray(W.T)

    in_maps = [
        {"X": X[i * NLOC:(i + 1) * NLOC], "Wt": Wt, "b": b} for i in range(NCORES)
    ]
    res = bass_utils.run_bass_kernel_spmd(
        nc, in_maps, core_ids=list(range(NCORES)), trace=trace, **kwargs
    )
    outp = np.concatenate([res.results[i]["out"] for i in range(NCORES)], axis=0)
    return outp, res


def kernel(**inputs):
    outp, _ = _run(inputs, trace=False)
    return outp

